# revision 51
# baseline (speedup 1.0000x reference)
"""Multi-head attention block (RMSNorm QK + RoPE + GQA + softmax + O-proj)
on 8 Trainium2 NeuronCores.

Sharding: data parallel over batch (B=2) x tensor parallel over kv-head
quarters (NKV=8 -> 2 kv heads / 4 q heads per core). Each core computes a
partial output [S, HID] = attn_out_local @ Wo_rows(local heads); the host
sums the 4 partials per batch (partials stored bf16, summed in f32).

All matmul operands are bf16 (fp32 PSUM accumulation): fp32r moving
operands stream at ~2 cycles/row on TRN2, bf16 at 1 cycle/row, so bf16
doubles PE throughput over the fp32r baseline.

Per-core pipeline:
  stage 1 (per s-tile): QKV projection with the hidden-state block as the
      stationary operand and weight blocks moving (all input DMAs on the
      two HWDGE rings: wq/tables/wo on the act queue, hsT -- split into
      sub-DMAs so early k-blocks land first -- and wkv on the sync queue;
      no SWDGE descriptor generation anywhere); RMSNorm sum-of-squares as one
      wide ACT Square + DVE inner-axis reduce per projection group; rstd
      applied by ACT Copy(scale=rstd[P,1]) producing f32 q/k; RoPE on DVE
      in f32 with head-wide ops (cos/sin tables stride-0-broadcast across
      heads), final add casting bf16; PE-transpose (bf16) into
      qTall/kTall [HD, S]; v copied s-layout into vSall.
  stage 2 (per q-chunk 512 x head): score groups [k,q] into [128,1024]
      PSUM tiles (2 k-tiles), one wide Exp per group on ACT
      (scale=1/sqrt(HD); RMSNorm bounds |scores| <= sqrt(HD), so no max
      subtraction); per group the PREVIOUS group's PV matmuls and one
      interleaved O-projection part run on the PE first so the score
      matmuls never catch the exp stream; softmax denominator: contiguous
      bf16 pair-sums of the exp groups on DVE, then one ones-matmul on the
      PE (~0.2us) whose PSUM output is the column sum replicated across
      all partitions; the denominator matmul and the normalization tail
      (reciprocal_approx_fast + one mul that fuses the PSUM->SBUF move)
      are deferred into the NEXT head's stream, as are the final PV
      matmuls, which kills the head-boundary bubbles; the previous
      chunk's O-projection is split into 16 four-matmul parts spread over
      this chunk's score slots; out rows stream as half-row DMAs.
"""

import math

import numpy as np

B, S, HID = 2, 2048, 2048
NH, NKV, HD = 16, 8, 128
EPS = 1e-6
THETA = 1000000.0
N_CORES = 8

P = 128
NT = S // P            # 16 s-tiles
KT = HID // P          # 16 hidden k-tiles
QC = 4                 # q chunks of 512
QW = S // QC           # 512
HEADS = NH // 4        # 4 q heads per core
KV = NKV // 4          # 2 kv heads per core
GRP = 2                # k-tiles per exp group (psc [P, GRP*QW] = 2 PSUM banks)
NG = NT // GRP         # 8 exp groups per (chunk, head)
GW = GRP * QW          # 1024
EC = 4                 # output e chunks of 512
EW = HID // EC         # 512

_CACHE = {}


def _build():
    if "nc" in _CACHE:
        return _CACHE["nc"]
    import concourse.tile as tile
    import concourse.mybir as mybir
    from concourse import bacc

    f32 = mybir.dt.float32
    bf16 = mybir.dt.bfloat16
    AF = mybir.ActivationFunctionType
    AX = mybir.AxisListType
    OP = mybir.AluOpType

    nc = bacc.Bacc("TRN2", target_bir_lowering=False, debug=False,
                   num_devices=N_CORES)

    hsT_d = nc.dram_tensor("hsT", [NT * P, KT * P], bf16, kind="ExternalInput").ap()
    wq_d = nc.dram_tensor("wq", [P, KT * HEADS * HD], bf16, kind="ExternalInput").ap()
    wkv_d = nc.dram_tensor("wkv", [P, KT * 2 * KV * HD], bf16, kind="ExternalInput").ap()
    wo_d = nc.dram_tensor("wo", [P, HEADS * HID], bf16, kind="ExternalInput").ap()
    cq_d = nc.dram_tensor("cosq", [P, NT * HD], f32, kind="ExternalInput").ap()
    sq_d = nc.dram_tensor("sinxq", [P, NT * HD], f32, kind="ExternalInput").ap()
    ck_d = nc.dram_tensor("cosk", [P, NT * HD], f32, kind="ExternalInput").ap()
    sk_d = nc.dram_tensor("sinxk", [P, NT * HD], f32, kind="ExternalInput").ap()
    id_d = nc.dram_tensor("ident", [P, P], bf16, kind="ExternalInput").ap()
    out_d = nc.dram_tensor("out", [S, HID], bf16, kind="ExternalOutput").ap()

    inv_sqrt_hd = 1.0 / math.sqrt(HD)
    NQ = HEADS * HD        # 512
    NKVW = 2 * KV * HD     # 512 (k 0:256 | v 256:512)
    NJ = HEADS + KV        # 6 normed+roped heads per s-tile
    H2 = HD // 2

    def bcast(ap_2d, n):
        # [P, F] -> [P, n, F] with stride-0 repeat along the middle dim
        return ap_2d.unsqueeze(1).broadcast_to([ap_2d.shape[0], n, ap_2d.shape[-1]])

    with tile.TileContext(nc) as tc:
        with tc.tile_pool(name="const", bufs=1) as cpool, \
             tc.tile_pool(name="qkv", bufs=1) as qkv_pool, \
             tc.tile_pool(name="ep0", bufs=1) as ep0, \
             tc.tile_pool(name="ps_sc_a", bufs=1, space="PSUM") as ps_sc_a, \
             tc.tile_pool(name="ps_pv_a", bufs=1, space="PSUM") as ps_pv_a:
            ident = cpool.tile([P, P], bf16)
            epsb = cpool.tile([P, 1], f32)
            ones = cpool.tile([P, P], bf16)
            nc.sync.dma_start(ident[:], id_d[:])
            nc.vector.memset(epsb[:], EPS)
            nc.vector.memset(ones[:], 1.0)

            qTall = qkv_pool.tile([P, HEADS * S], bf16)
            kTall = qkv_pool.tile([P, KV * S], bf16)
            vSall = qkv_pool.tile([P, KV * S], bf16)
            wo_sb = qkv_pool.tile([P, HEADS * HID], bf16)

            qT_r = qTall.rearrange("p (h s) -> p h s", h=HEADS)
            kT_r = kTall.rearrange("p (h s) -> p h s", h=KV)
            vS_r = vSall.rearrange("p (h s) -> p h s", h=KV)

            # ---------------- stage 1: projections + norm + rope ----------
            with tc.tile_pool(name="w1", bufs=1) as w1, \
                 tc.tile_pool(name="tbl", bufs=1) as tbl, \
                 tc.tile_pool(name="hst", bufs=3) as hst, \
                 tc.tile_pool(name="rp", bufs=2) as rp, \
                 tc.tile_pool(name="sm", bufs=8) as sm, \
                 tc.tile_pool(name="ps_qkv", bufs=2, space="PSUM") as ps_qkv, \
                 tc.tile_pool(name="ps_tr", bufs=1, space="PSUM") as ps_tr:
                wq_sb = w1.tile([P, KT * NQ], bf16)
                wkv_sb = w1.tile([P, KT * NKVW], bf16)
                cq_sb = tbl.tile([P, NT * HD], f32)
                sq_sb = tbl.tile([P, NT * HD], f32)
                ck_sb = tbl.tile([P, NT * HD], f32)
                sk_sb = tbl.tile([P, NT * HD], f32)
                # DMA layout. HWDGE dma_starts OCCUPY the issuing engine for
                # the whole transfer, so bulk traffic stays on the SWDGE
                # (gpsimd) path; but SWDGE has a ~9us cold-start, so the
                # tile-0-critical pieces ride the two HWDGE rings, placed
                # before any engine compute:
                #   sync ring: ident, hsT0 head, wq per-k, rope tables, wo,
                #              then stage-2 out rows
                #   act ring : wkv per-k (ACT is idle until tile 0's rope,
                #              ~14us in, so blocking it early is free)
                #   swdge    : rest of hsT0, hsT tiles 1..15
                hsT0 = hst.tile([P, HID], bf16, tag="hsTt", name="hsT0")
                nc.sync.dma_start(hsT0[:, 0:2 * P], hsT_d[0:P, 0:2 * P])
                # the first two wq blocks ride the act ring so the very first
                # matmul's operands arrive on two parallel rings
                for k in range(2):
                    nc.scalar.dma_start(wq_sb[:, k * NQ:(k + 1) * NQ],
                                        wq_d[:, k * NQ:(k + 1) * NQ])
                for k in range(KT):
                    nc.scalar.dma_start(wkv_sb[:, k * NKVW:(k + 1) * NKVW],
                                        wkv_d[:, k * NKVW:(k + 1) * NKVW])
                nc.gpsimd.dma_start(hsT0[:, 2 * P:8 * P], hsT_d[0:P, 2 * P:8 * P])
                nc.gpsimd.dma_start(hsT0[:, 8 * P:], hsT_d[0:P, 8 * P:])
                for k in range(2, KT):
                    nc.sync.dma_start(wq_sb[:, k * NQ:(k + 1) * NQ],
                                      wq_d[:, k * NQ:(k + 1) * NQ])
                for k in range(KT):
                    sl = slice(k * HD, (k + 1) * HD)
                    nc.sync.dma_start(cq_sb[:, sl], cq_d[:, sl])
                    nc.sync.dma_start(sq_sb[:, sl], sq_d[:, sl])
                    nc.sync.dma_start(ck_sb[:, sl], ck_d[:, sl])
                    nc.sync.dma_start(sk_sb[:, sl], sk_d[:, sl])
                nc.sync.dma_start(wo_sb[:], wo_d[:])

                def flush_tr(st):
                    t_ = st["t"]
                    # vS copy first: it releases the pskv PSUM buffer and has
                    # no transpose dependency
                    nc.vector.tensor_copy(
                        vS_r[:, :, t_ * P:(t_ + 1) * P],
                        st["pskv"][:, KV * HD:2 * KV * HD].rearrange(
                            "p (h s) -> p h s", h=KV))
                    ptr = ps_tr.tile([P, NJ * P], bf16, tag="ptr", name="ptr")
                    for grp, nh, j0 in (("q", HEADS, 0), ("k", KV, HEADS)):
                        qr = st["qrots"][grp]
                        for i in range(nh):
                            # transpose-mode matmul: the only PE path that can
                            # write bf16 to PSUM (keeps ptr at 1 bank and the
                            # DVE copies in 2x mode)
                            nc.tensor.transpose(
                                ptr[:, (j0 + i) * P:(j0 + i + 1) * P],
                                qr[:, i * HD:(i + 1) * HD], ident[:])
                    nc.vector.tensor_copy(
                        qT_r[:, :, t_ * P:(t_ + 1) * P],
                        ptr[:, 0:HEADS * P].rearrange("p (h s) -> p h s", h=HEADS))
                    nc.vector.tensor_copy(
                        kT_r[:, :, t_ * P:(t_ + 1) * P],
                        ptr[:, HEADS * P:NJ * P].rearrange("p (h s) -> p h s", h=KV))

                # early attention for head (c0,h0): score group g needs only
                # k-tiles 2g,2g+1 (flushed by tile 2g+2's k==12 slot) and qT
                # tiles 0..3 (flushed by tile 4), so groups 0..6 are emitted
                # INSIDE the t-loop right after the flush slot, where the two
                # extra matmuls ride the PE-bound stream and the exps hide
                # under ACT's large stage-1 slack. This starts the stage-2
                # exp stream (chunk 0's pacer) a full head early and leaves
                # no serial exp ping-pong at the stage boundary.
                e_early = ep0.tile([P, NG * GW], bf16, name="e_early")
                # head (c0,h1)'s first four groups ride the remaining odd
                # tiles; its last four run normally in stage 2, halving chunk
                # 0's ACT-exp deficit
                e_early2 = ep0.tile([P, 4 * GW], bf16, name="e_early2")
                EARLY_AT = {5: (0, 0), 6: (0, 1), 7: (0, 2), 8: (0, 3),
                            10: (0, 4), 12: (0, 5), 14: (0, 6),
                            9: (1, 0), 11: (1, 1), 13: (1, 2), 15: (1, 3)}

                def early_score(hh, g):
                    psc = ps_sc_a.tile([P, GW], f32, tag="psc", name="psca")
                    dst = e_early if hh == 0 else e_early2
                    for i in range(GRP):
                        kk = g * GRP + i
                        nc.tensor.matmul(
                            psc[:, i * QW:(i + 1) * QW],
                            kT_r[:, 0, kk * P:(kk + 1) * P],
                            qT_r[:, hh, 0:QW], start=True, stop=True)
                    nc.scalar.activation(dst[:, g * GW:(g + 1) * GW],
                                         psc[:], AF.Exp, scale=inv_sqrt_hd)

                pend_tr = None
                for t in range(NT):
                    if t == 0:
                        hsTt = hsT0
                    else:
                        hsTt = hst.tile([P, HID], bf16, tag="hsTt", name="hsTt")
                        nc.gpsimd.dma_start(hsTt[:], hsT_d[t * P:(t + 1) * P, :])
                    psq = ps_qkv.tile([P, NQ], f32, tag="psq")
                    pskv = ps_qkv.tile([P, NKVW], f32, tag="pskv")
                    for k in range(KT):
                        if k == 12 and pend_tr is not None:
                            # previous tile's transposes + copies, deferred
                            # into this tile's matmul stream; k==12 gives the
                            # rope chain ~5us of slack (at k==8 the first
                            # transpose stalled ~1us/tile on the rope adds)
                            flush_tr(pend_tr)
                            pend_tr = None
                        if k == 13 and t in EARLY_AT:
                            early_score(*EARLY_AT[t])
                        hk = hsTt[:, k * P:(k + 1) * P]
                        nc.tensor.matmul(psq[:], hk, wq_sb[:, k * NQ:(k + 1) * NQ],
                                         start=(k == 0), stop=(k == KT - 1))
                        nc.tensor.matmul(pskv[:], hk, wkv_sb[:, k * NKVW:(k + 1) * NKVW],
                                         start=(k == 0), stop=(k == KT - 1))

                    # sum-of-squares: one wide Square per projection group,
                    # then an inner-axis DVE reduce per head
                    sqq = rp.tile([P, NQ], f32, tag="sqq", name="sqq")
                    nc.scalar.activation(sqq[:], psq[:], AF.Square)
                    sqk = rp.tile([P, KV * HD], f32, tag="sqk", name="sqk")
                    nc.scalar.activation(sqk[:], pskv[:, 0:KV * HD], AF.Square)
                    sumsq = sm.tile([P, 8], f32, tag="sumsq")
                    nc.vector.tensor_reduce(
                        sumsq[:, 0:HEADS],
                        sqq.rearrange("p (h d) -> p h d", h=HEADS),
                        AX.X, OP.add)
                    nc.vector.tensor_reduce(
                        sumsq[:, HEADS:NJ],
                        sqk.rearrange("p (h d) -> p h d", h=KV),
                        AX.X, OP.add)
                    std = sm.tile([P, 8], f32, tag="std")
                    nc.scalar.activation(std[:, 0:NJ], sumsq[:, 0:NJ], AF.Sqrt,
                                         scale=1.0 / HD, bias=epsb[:])
                    rstd = sm.tile([P, 8], f32, tag="rstd")
                    nc.vector.reciprocal(rstd[:, 0:NJ], std[:, 0:NJ])

                    qrots = {}
                    # wide RoPE: q heads [P, 4*HD] then kv heads [P, 2*HD]
                    for grp, nh, j0, cos_sb, sin_sb in (
                            ("q", HEADS, 0, cq_sb, sq_sb),
                            ("k", KV, HEADS, ck_sb, sk_sb)):
                        W = nh * HD
                        src = psq if grp == "q" else pskv
                        qn = rp.tile([P, HEADS * HD], f32, tag=f"qn{grp}",
                                     name="qn")
                        # apply rstd as ONE DVE mul with the per-head scale
                        # stride-0-broadcast along d (vs 6 serial ~460ns ACT
                        # copies: shortens the rope-chain latency ~1.7us)
                        rstd_b = rstd[:, j0:j0 + nh].unsqueeze(2).broadcast_to(
                            [P, nh, HD])
                        nc.vector.tensor_mul(
                            qn[:, 0:W].rearrange("p (h d) -> p h d", h=nh),
                            src[:, 0:W].rearrange("p (h d) -> p h d", h=nh),
                            rstd_b)
                        cos_t = cos_sb[:, t * HD:(t + 1) * HD]
                        sin_t = sin_sb[:, t * HD:(t + 1) * HD]
                        qn_r = qn[:, 0:W].rearrange("p (h d) -> p h d", h=nh)
                        t1 = rp.tile([P, HEADS * HD], f32, tag=f"t1{grp}",
                                     name="t1")
                        nc.vector.tensor_mul(t1[:, 0:W], qn[:, 0:W], bcast(cos_t, nh))
                        t2 = rp.tile([P, HEADS * HD], f32, tag=f"t2{grp}",
                                     name="t2")
                        t2_r = t2[:, 0:W].rearrange("p (h d) -> p h d", h=nh)
                        nc.vector.tensor_mul(t2_r[:, :, 0:H2], qn_r[:, :, H2:HD],
                                             bcast(sin_t[:, 0:H2], nh))
                        nc.vector.tensor_mul(t2_r[:, :, H2:HD], qn_r[:, :, 0:H2],
                                             bcast(sin_t[:, H2:HD], nh))
                        qrot = rp.tile([P, HEADS * HD], bf16, tag=f"qr{grp}",
                                       name="qrot")
                        nc.vector.tensor_add(qrot[:, 0:W], t1[:, 0:W], t2[:, 0:W])
                        qrots[grp] = qrot
                    pend_tr = {"t": t, "qrots": qrots, "pskv": pskv}

                # the early head's PV accumulation for groups 0..6 (vS tiles
                # 0..13 are flushed): pure PE filler that covers tile 15's
                # rope-chain latency before its transposes can run
                ppv_early = ps_pv_a.tile([P, QW], f32, tag="ppv",
                                         name="ppv_early")
                for g in range(7):
                    for i in range(GRP):
                        kk = g * GRP + i
                        nc.tensor.matmul(
                            ppv_early[:], vS_r[:, 0, kk * P:(kk + 1) * P],
                            e_early[:, g * GW + i * QW:g * GW + (i + 1) * QW],
                            start=(kk == 0), stop=False)
                flush_tr(pend_tr)
                pend_tr = None
                # group 7 needs tile 15's kT, so it follows the last flush
                early_score(0, 7)

            # ---------------- stage 2: attention + O-projection -----------
            with tc.tile_pool(name="ep", bufs=2) as ep, \
                 tc.tile_pool(name="ac", bufs=2) as ac, \
                 tc.tile_pool(name="dn", bufs=2) as dn, \
                 tc.tile_pool(name="on", bufs=8) as on, \
                 tc.tile_pool(name="ob", bufs=3) as ob, \
                 tc.tile_pool(name="ps_sc", bufs=1, space="PSUM") as ps_sc, \
                 tc.tile_pool(name="ps_pv", bufs=1, space="PSUM") as ps_pv, \
                 tc.tile_pool(name="ps_o", bufs=2, space="PSUM") as ps_o:

                wo_state = {"row": None}

                def wo_part(c, onT_c, p, final=False):
                    # one [P, EW] slice of the previous chunk's O-projection:
                    # 4 accumulating matmuls + a PSUM->SBUF copy (+ DMA on
                    # row completion). Interleaved between score groups so
                    # the exp stream never starves.
                    qt, e_ = divmod(p, EC)
                    if e_ == 0:
                        wo_state["row"] = ob.tile([P, HID], bf16, tag="outb",
                                                  name="out_row")
                    out_row = wo_state["row"]
                    pso = ps_o.tile([P, EW], f32, tag="pso", name="pso")
                    for h in range(HEADS):
                        nc.tensor.matmul(
                            pso[:],
                            onT_c[h][:, qt * P:(qt + 1) * P],
                            wo_sb[:, h * HID + e_ * EW: h * HID + (e_ + 1) * EW],
                            start=(h == 0), stop=(h == HEADS - 1))
                    if final and e_ % 2 == 0:
                        # at the end-of-kernel drain ACT is idle: splitting
                        # the copies across both engines doubles throughput
                        nc.scalar.activation(out_row[:, e_ * EW:(e_ + 1) * EW],
                                             pso[:], AF.Copy)
                    else:
                        nc.vector.tensor_copy(out_row[:, e_ * EW:(e_ + 1) * EW],
                                              pso[:])
                    # stream each half-row as soon as its parts are done so
                    # the end-of-kernel DMA tail is half as long; the drain
                    # chunk goes out in quarters
                    rows = out_d[c * QW + qt * P: c * QW + (qt + 1) * P, :]
                    if e_ == 1:
                        nc.sync.dma_start(rows[:, 0:2 * EW],
                                          out_row[:, 0:2 * EW])
                    elif final and e_ >= 2:
                        nc.sync.dma_start(rows[:, e_ * EW:(e_ + 1) * EW],
                                          out_row[:, e_ * EW:(e_ + 1) * EW])
                    elif e_ == EC - 1:
                        nc.sync.dma_start(rows[:, 2 * EW:],
                                          out_row[:, 2 * EW:])

                def emit_wo(c, onT_c):
                    for p in range(QC * EC):
                        wo_part(c, onT_c, p, final=True)

                def emit_den(st):
                    # cross-partition reduce of the per-partition denominator
                    # tree root as a single ones-matmul (~0.2us on the PE, vs
                    # 3.3us for the old GPSIMD partition_all_reduce); every
                    # output partition receives the same column sum, so the
                    # result lands pre-replicated in PSUM. Shares the ps_o
                    # buffer rotation to stay within the 8 PSUM banks.
                    pden = ps_o.tile([P, QW], f32, tag="pso", name="pden")
                    nc.tensor.matmul(pden[:], ones[:], st["root"][:],
                                     start=True, stop=True)
                    st["den"] = pden

                def emit_tail(st):
                    # deferred normalization tail for a finished head:
                    # reciprocal of the broadcast denominator, then one mul
                    # that both normalizes and moves ppv PSUM->SBUF.
                    rcp = dn.tile([P, QW], f32, tag="rcp", name="rcp")
                    nc.vector.reciprocal_approx_fast(rcp[:], st["den"][:])
                    onT = on.tile([P, QW], bf16, tag="onT", name="onT")
                    nc.vector.tensor_mul(onT[:], st["ppv"][:], rcp[:])
                    st["onT_c"].append(onT)

                # O-projection part slots (head, group). Head-START slots
                # (h,0) fill the boundary where the next head's first score
                # matmul waits on the lagging exp stream; h0 only gets late
                # slots (g5/g7) because the previous chunk's h3 tail -- which
                # completes onT_c -- lands at (h0, g3).
                part_slots = {(0, 5), (0, 7), (1, 0), (2, 0), (3, 0)} | {
                    (h, g) for h in range(1, HEADS) for g in (1, 3, 5, 7)}

                prev_wo = None
                prev_parts = 0
                prev_tail = None
                prev_pv = None
                for c in range(QC):
                    onT_c = []
                    for h in range(HEADS):
                        # head (0,0)'s scores, exps and PV groups 0..6
                        # already ran during stage 1 (e_early / ppv_early);
                        # head (0,1)'s first four score groups are in e_early2
                        early = (c == 0 and h == 0)
                        half2 = (c == 0 and h == 1)
                        last = (c == QC - 1 and h == HEADS - 1)
                        kv = h // (HEADS // KV)
                        hidx = c * HEADS + h
                        # heads alternate between the hoisted 1-buf pool and
                        # the stage-2 one (= the old bufs=2 rotation)
                        ppv = ppv_early if early else (
                            ps_pv_a if hidx % 2 == 0 else ps_pv).tile(
                                [P, QW], f32, tag="ppv", name="ppv")
                        # all 8 exp groups of this head live in one wide tile
                        # so the denominator tree runs as 4 strided DVE ops
                        e_all = e_early if early else ep.tile(
                            [P, NG * GW], bf16, tag="e", name="e_all")

                        def esrc(g):
                            # which tile holds this head's exp group g
                            return e_early2 if (half2 and g < 4) else e_all

                        def pv_of(g, dst):
                            src = esrc(g)
                            for i in range(GRP):
                                kk = g * GRP + i
                                nc.tensor.matmul(
                                    dst[:], vS_r[:, kv, kk * P:(kk + 1) * P],
                                    src[:, g * GW + i * QW:g * GW + (i + 1) * QW],
                                    start=(kk == 0), stop=(kk == NT - 1))

                        halfsums = []

                        def den_half(lo):
                            # sum exp groups [lo, lo+4) pairwise with fully
                            # contiguous [P, 2048] operands (strided views
                            # break the DVE fast path); groups lo..lo+3 always
                            # live in one tile (esrc is constant over them)
                            src = esrc(lo)
                            hs_ = ac.tile([P, 2 * GW], bf16, tag="hsum",
                                          name="hs_")
                            nc.vector.tensor_add(
                                hs_[:], src[:, lo * GW:(lo + 2) * GW],
                                src[:, (lo + 2) * GW:(lo + 4) * GW])
                            halfsums.append(hs_)

                        for g in range(NG):
                            emit_sc = not early and not (half2 and g < 4)
                            if emit_sc:
                                # alternate between the hoisted 2-bank pool
                                # and the stage-2 one (= the old bufs=2 ring)
                                psc = (ps_sc_a if g % 2 == 0 else ps_sc).tile(
                                    [P, GW], f32, tag="psc", name="psc")
                            # PE work for the PREVIOUS group goes first so the
                            # score matmuls never catch up with the exp stream
                            if g >= 1:
                                if g == 1 and prev_pv is not None:
                                    # previous head's trailing PV, deferred two
                                    # score groups so its exp has completed
                                    prev_pv()
                                    prev_pv = None
                                if not early:
                                    # (the early head's groups 0..6 were
                                    # accumulated into ppv_early in stage 1,
                                    # and its g7 rides prev_pv)
                                    pv_of(g - 1, ppv)
                            if g == 2 and prev_tail is not None:
                                # previous head's denominator matmul, deferred
                                # far enough that its DVE tree has completed
                                emit_den(prev_tail)
                            if g == 3 and prev_tail is not None:
                                # previous head's normalization, deferred far
                                # enough that its all-reduce has completed
                                emit_tail(prev_tail)
                                prev_tail = None
                            if (prev_wo is not None and (h, g) in part_slots
                                    and prev_parts < QC * EC):
                                wo_part(prev_wo[0], prev_wo[1], prev_parts)
                                prev_parts += 1
                            if emit_sc:
                                for i in range(GRP):
                                    kk = g * GRP + i
                                    nc.tensor.matmul(
                                        psc[:, i * QW:(i + 1) * QW],
                                        kT_r[:, kv, kk * P:(kk + 1) * P],
                                        qT_r[:, h, c * QW:(c + 1) * QW],
                                        start=True, stop=True)
                                nc.scalar.activation(
                                    e_all[:, g * GW:(g + 1) * GW], psc[:],
                                    AF.Exp, scale=inv_sqrt_hd)
                            if g == 3:
                                den_half(0)
                            if last and g == 6:
                                # pre-fold everything not depending on exp6/7
                                # so the end-of-kernel tail after the final
                                # exp is as short as possible
                                dq45 = ac.tile([P, GW], bf16, tag="dq45",
                                               name="dq45")
                                nc.vector.tensor_add(dq45[:],
                                                     e_all[:, 4 * GW:5 * GW],
                                                     e_all[:, 5 * GW:6 * GW])
                                fold0 = ac.tile([P, GW], bf16, tag="fold0",
                                                name="fold0")
                                nc.vector.tensor_add(fold0[:],
                                                     halfsums[0][:, 0:GW],
                                                     halfsums[0][:, GW:2 * GW])
                                r0 = ac.tile([P, QW], bf16, tag="r0", name="r0")
                                nc.vector.tensor_add(r0[:], fold0[:, 0:QW],
                                                     fold0[:, QW:GW])
                        if last:
                            # short-critical-path tree: only ~1.8us of DVE
                            # work remains after the final exp
                            dq67 = ac.tile([P, GW], bf16, tag="dq67",
                                           name="dq67")
                            nc.vector.tensor_add(dq67[:],
                                                 e_all[:, 6 * GW:7 * GW],
                                                 e_all[:, 7 * GW:8 * GW])
                            s2 = ac.tile([P, GW], bf16, tag="s2", name="s2")
                            nc.vector.tensor_add(s2[:], dq45[:], dq67[:])
                            r1 = ac.tile([P, QW], bf16, tag="r1", name="r1")
                            nc.vector.tensor_add(r1[:], s2[:, 0:QW],
                                                 s2[:, QW:GW])
                            root = ac.tile([P, QW], bf16, tag="root",
                                           name="root")
                            nc.vector.tensor_add(root[:], r0[:], r1[:])
                        else:
                            # denominator: finish the strided bf16 tree, fold
                            # (the cross-partition reduce is the ones-matmul
                            # in emit_den)
                            den_half(4)
                            qsum = ac.tile([P, 2 * GW], bf16, tag="qsum",
                                           name="qsum")
                            nc.vector.tensor_add(qsum[:], halfsums[0][:],
                                                 halfsums[1][:])
                            wsum = ac.tile([P, GW], bf16, tag="wsum",
                                           name="wsum")
                            nc.vector.tensor_add(wsum[:], qsum[:, 0:GW],
                                                 qsum[:, GW:2 * GW])
                            root = ac.tile([P, QW], bf16, tag="root",
                                           name="root")
                            nc.vector.tensor_add(root[:], wsum[:, 0:QW],
                                                 wsum[:, QW:2 * QW])
                        prev_pv = (lambda gg=NG - 1, dst=ppv, ea=e_all, kvv=kv:
                                   [nc.tensor.matmul(
                                       dst[:], vS_r[:, kvv, (gg * GRP + i) * P:
                                                    (gg * GRP + i + 1) * P],
                                       ea[:, gg * GW + i * QW:gg * GW + (i + 1) * QW],
                                       start=False, stop=(i == GRP - 1))
                                    for i in range(GRP)])
                        prev_tail = {"root": root, "ppv": ppv, "onT_c": onT_c}

                    assert prev_wo is None or prev_parts == QC * EC, prev_parts
                    prev_wo = (c, onT_c)
                    prev_parts = 0
                prev_pv()
                prev_pv = None
                emit_den(prev_tail)
                emit_tail(prev_tail)
                prev_tail = None
                emit_wo(*prev_wo)

    nc.compile()
    _CACHE["nc"] = nc
    return nc


def _host_prep(hidden_states, position_ids, Wq, Wk, Wv, Wo, q_norm_w, k_norm_w):
    """Build the 8 per-core input maps (bf16 matmul operands, pre-tiled)."""
    import ml_dtypes
    bf = ml_dtypes.bfloat16

    hidden_states = np.asarray(hidden_states, dtype=np.float32)
    Wq = np.asarray(Wq, dtype=np.float32)
    Wk = np.asarray(Wk, dtype=np.float32)
    Wv = np.asarray(Wv, dtype=np.float32)
    Wo = np.asarray(Wo, dtype=np.float32)
    q_norm_w = np.asarray(q_norm_w, dtype=np.float32)
    k_norm_w = np.asarray(k_norm_w, dtype=np.float32)
    pos = np.asarray(position_ids)

    ident = np.eye(P, dtype=bf)

    # per-batch rope tables with sign fold and norm-weight fold, pre-tiled
    # to [p, (t, d)] so each table is a single contiguous DMA
    inv_freq = (1.0 / THETA ** (np.arange(0, HD, 2, dtype=np.float32) / HD)
                ).astype(np.float32)
    H2 = HD // 2

    def tile_tab(x):  # [S, HD] -> [P, NT*HD] f32
        return np.ascontiguousarray(
            x.reshape(NT, P, HD).transpose(1, 0, 2).reshape(P, NT * HD))

    tabs = []
    for b in range(B):
        freqs = pos[b].astype(np.float32)[:, None] * inv_freq[None, :]
        emb = np.concatenate([freqs, freqs], axis=-1)          # [S, HD]
        cos = np.cos(emb).astype(np.float32)
        sin = np.sin(emb).astype(np.float32)
        sinx = sin.copy()
        sinx[:, :H2] *= -1.0
        wq_sw = np.concatenate([q_norm_w[H2:], q_norm_w[:H2]])
        wk_sw = np.concatenate([k_norm_w[H2:], k_norm_w[:H2]])
        tabs.append({
            "cosq": tile_tab(cos * q_norm_w[None, :]),
            "sinxq": tile_tab(sinx * wq_sw[None, :]),
            "cosk": tile_tab(cos * k_norm_w[None, :]),
            "sinxk": tile_tab(sinx * wk_sw[None, :]),
        })

    # Pre-tiled transpose: hsT_t[t*P+p, k*P+c] = hs[b][t*P+c, k*P+p] so each
    # s-tile's SBUF load is a plain [P, HID] slice with 4KB-contiguous rows.
    hsT = []
    for b in range(B):
        x = hidden_states[b].reshape(NT, P, KT, P)      # [t, c, k, p]
        x = np.ascontiguousarray(x.transpose(0, 3, 2, 1))  # [t, p, k, c]
        hsT.append(x.reshape(NT * P, KT * P).astype(bf))

    def tile_w(w):  # [HID, N] -> [P, KT*N]
        n = w.shape[1]
        return np.ascontiguousarray(
            w.reshape(KT, P, n).transpose(1, 0, 2).reshape(P, KT * n)).astype(bf)

    def tile_wo(w):  # [HEADS*HD, HID] -> [P, HEADS*HID]
        return np.ascontiguousarray(
            w.reshape(HEADS, P, HID).transpose(1, 0, 2).reshape(P, HEADS * HID)
        ).astype(bf)

    in_maps = []
    for c in range(N_CORES):
        b = c // 4
        q = c % 4
        qs = slice(q * HEADS * HD, (q + 1) * HEADS * HD)
        ks = slice(q * KV * HD, (q + 1) * KV * HD)
        in_maps.append({
            "hsT": hsT[b],
            "wq": tile_w(Wq[:, qs]),
            "wkv": tile_w(np.concatenate([Wk[:, ks], Wv[:, ks]], axis=1)),
            "wo": tile_wo(Wo[qs, :]),
            "cosq": tabs[b]["cosq"],
            "sinxq": tabs[b]["sinxq"],
            "cosk": tabs[b]["cosk"],
            "sinxk": tabs[b]["sinxk"],
            "ident": ident,
        })
    return in_maps


def _gather(results):
    out = np.empty((B, S, HID), dtype=np.float32)
    for b in range(B):
        acc = results[4 * b]["out"].astype(np.float32)
        for i in range(1, 4):
            acc = acc + results[4 * b + i]["out"].astype(np.float32)
        out[b] = acc
    return out


def kernel(hidden_states, position_ids, Wq, Wk, Wv, Wo, q_norm_w, k_norm_w,
           _trace=False):
    from concourse.bass_utils import run_bass_kernel_spmd

    nc = _build()
    in_maps = _host_prep(hidden_states, position_ids, Wq, Wk, Wv, Wo,
                         q_norm_w, k_norm_w)
    res = run_bass_kernel_spmd(nc, in_maps, core_ids=list(range(N_CORES)),
                               trace=_trace)
    out = _gather(res.results)
    if _trace:
        kernel.last_result = res
    return out



# revision 53
# speedup vs baseline: 1.0190x; 1.0190x over previous
"""Multi-head attention block (RMSNorm QK + RoPE + GQA + softmax + O-proj)
on 8 Trainium2 NeuronCores.

Sharding: data parallel over batch (B=2) x tensor parallel over kv-head
quarters (NKV=8 -> 2 kv heads / 4 q heads per core). Each core computes a
partial output [S, HID] = attn_out_local @ Wo_rows(local heads); the host
sums the 4 partials per batch (partials stored bf16, summed in f32).

All matmul operands are bf16 (fp32 PSUM accumulation): fp32r moving
operands stream at ~2 cycles/row on TRN2, bf16 at 1 cycle/row, so bf16
doubles PE throughput over the fp32r baseline.

Per-core pipeline:
  stage 1 (per s-tile): QKV projection with the hidden-state block as the
      stationary operand and weight blocks moving (all input DMAs on the
      two HWDGE rings: wq/tables/wo on the act queue, hsT -- split into
      sub-DMAs so early k-blocks land first -- and wkv on the sync queue;
      no SWDGE descriptor generation anywhere); RMSNorm sum-of-squares as one
      wide ACT Square + DVE inner-axis reduce per projection group; rstd
      applied by ACT Copy(scale=rstd[P,1]) producing f32 q/k; RoPE on DVE
      in f32 with head-wide ops (cos/sin tables stride-0-broadcast across
      heads), final add casting bf16; PE-transpose (bf16) into
      qTall/kTall [HD, S]; v copied s-layout into vSall.
  stage 2 (per q-chunk 512 x head): score groups [k,q] into [128,1024]
      PSUM tiles (2 k-tiles), one wide Exp per group on ACT
      (scale=1/sqrt(HD); RMSNorm bounds |scores| <= sqrt(HD), so no max
      subtraction); per group the PREVIOUS group's PV matmuls and one
      interleaved O-projection part run on the PE first so the score
      matmuls never catch the exp stream; softmax denominator: contiguous
      bf16 pair-sums of the exp groups on DVE, then one ones-matmul on the
      PE (~0.2us) whose PSUM output is the column sum replicated across
      all partitions; the denominator matmul and the normalization tail
      (reciprocal_approx_fast + one mul that fuses the PSUM->SBUF move)
      are deferred into the NEXT head's stream, as are the final PV
      matmuls, which kills the head-boundary bubbles; the previous
      chunk's O-projection is split into 16 four-matmul parts spread over
      this chunk's score slots; out rows stream as half-row DMAs.
"""

import math

import numpy as np

B, S, HID = 2, 2048, 2048
NH, NKV, HD = 16, 8, 128
EPS = 1e-6
THETA = 1000000.0
N_CORES = 8

P = 128
NT = S // P            # 16 s-tiles
KT = HID // P          # 16 hidden k-tiles
QC = 4                 # q chunks of 512
QW = S // QC           # 512
HEADS = NH // 4        # 4 q heads per core
KV = NKV // 4          # 2 kv heads per core
GRP = 2                # k-tiles per exp group (psc [P, GRP*QW] = 2 PSUM banks)
NG = NT // GRP         # 8 exp groups per (chunk, head)
GW = GRP * QW          # 1024
EC = 4                 # output e chunks of 512
EW = HID // EC         # 512

_CACHE = {}


def _build():
    if "nc" in _CACHE:
        return _CACHE["nc"]
    import concourse.tile as tile
    import concourse.mybir as mybir
    from concourse import bacc

    f32 = mybir.dt.float32
    bf16 = mybir.dt.bfloat16
    AF = mybir.ActivationFunctionType
    AX = mybir.AxisListType
    OP = mybir.AluOpType

    nc = bacc.Bacc("TRN2", target_bir_lowering=False, debug=False,
                   num_devices=N_CORES)

    hsT_d = nc.dram_tensor("hsT", [NT * P, KT * P], bf16, kind="ExternalInput").ap()
    wq_d = nc.dram_tensor("wq", [P, KT * HEADS * HD], bf16, kind="ExternalInput").ap()
    wkv_d = nc.dram_tensor("wkv", [P, KT * 2 * KV * HD], bf16, kind="ExternalInput").ap()
    wo_d = nc.dram_tensor("wo", [P, HEADS * HID], bf16, kind="ExternalInput").ap()
    cq_d = nc.dram_tensor("cosq", [P, NT * HD], f32, kind="ExternalInput").ap()
    sq_d = nc.dram_tensor("sinxq", [P, NT * HD], f32, kind="ExternalInput").ap()
    ck_d = nc.dram_tensor("cosk", [P, NT * HD], f32, kind="ExternalInput").ap()
    sk_d = nc.dram_tensor("sinxk", [P, NT * HD], f32, kind="ExternalInput").ap()
    id_d = nc.dram_tensor("ident", [P, P], bf16, kind="ExternalInput").ap()
    out_d = nc.dram_tensor("out", [S, HID], bf16, kind="ExternalOutput").ap()

    inv_sqrt_hd = 1.0 / math.sqrt(HD)
    NQ = HEADS * HD        # 512
    NKVW = 2 * KV * HD     # 512 (k 0:256 | v 256:512)
    NJ = HEADS + KV        # 6 normed+roped heads per s-tile
    H2 = HD // 2

    def bcast(ap_2d, n):
        # [P, F] -> [P, n, F] with stride-0 repeat along the middle dim
        return ap_2d.unsqueeze(1).broadcast_to([ap_2d.shape[0], n, ap_2d.shape[-1]])

    with tile.TileContext(nc) as tc:
        with tc.tile_pool(name="const", bufs=1) as cpool, \
             tc.tile_pool(name="qkv", bufs=1) as qkv_pool, \
             tc.tile_pool(name="ep0", bufs=1) as ep0, \
             tc.tile_pool(name="ps_sc_a", bufs=1, space="PSUM") as ps_sc_a, \
             tc.tile_pool(name="ps_pv_a", bufs=1, space="PSUM") as ps_pv_a:
            ident = cpool.tile([P, P], bf16)
            epsb = cpool.tile([P, 1], f32)
            ones = cpool.tile([P, P], bf16)
            nc.sync.dma_start(ident[:], id_d[:])
            nc.vector.memset(epsb[:], EPS)
            nc.vector.memset(ones[:], 1.0)

            qTall = qkv_pool.tile([P, HEADS * S], bf16)
            kTall = qkv_pool.tile([P, KV * S], bf16)
            vSall = qkv_pool.tile([P, KV * S], bf16)
            wo_sb = qkv_pool.tile([P, HEADS * HID], bf16)

            qT_r = qTall.rearrange("p (h s) -> p h s", h=HEADS)
            kT_r = kTall.rearrange("p (h s) -> p h s", h=KV)
            vS_r = vSall.rearrange("p (h s) -> p h s", h=KV)

            # ---------------- stage 1: projections + norm + rope ----------
            with tc.tile_pool(name="w1", bufs=1) as w1, \
                 tc.tile_pool(name="tbl", bufs=1) as tbl, \
                 tc.tile_pool(name="hst", bufs=3) as hst, \
                 tc.tile_pool(name="rp", bufs=2) as rp, \
                 tc.tile_pool(name="sm", bufs=8) as sm, \
                 tc.tile_pool(name="ps_qkv", bufs=2, space="PSUM") as ps_qkv, \
                 tc.tile_pool(name="ps_tr", bufs=1, space="PSUM") as ps_tr:
                wq_sb = w1.tile([P, KT * NQ], bf16)
                wkv_sb = w1.tile([P, KT * NKVW], bf16)
                cq_sb = tbl.tile([P, NT * HD], f32)
                sq_sb = tbl.tile([P, NT * HD], f32)
                ck_sb = tbl.tile([P, NT * HD], f32)
                sk_sb = tbl.tile([P, NT * HD], f32)
                # DMA layout. HWDGE dma_starts OCCUPY the issuing engine for
                # the whole transfer, so bulk traffic stays on the SWDGE
                # (gpsimd) path; but SWDGE has a ~9us cold-start, so the
                # tile-0-critical pieces ride the two HWDGE rings, placed
                # before any engine compute:
                #   sync ring: ident, hsT0 head, wq per-k, rope tables, wo,
                #              then stage-2 out rows
                #   act ring : wkv per-k (ACT is idle until tile 0's rope,
                #              ~14us in, so blocking it early is free)
                #   swdge    : rest of hsT0, hsT tiles 1..15
                hsT0 = hst.tile([P, HID], bf16, tag="hsTt", name="hsT0")
                nc.sync.dma_start(hsT0[:, 0:2 * P], hsT_d[0:P, 0:2 * P])
                # the first two wq blocks ride the act ring so the very first
                # matmul's operands arrive on two parallel rings
                for k in range(2):
                    nc.scalar.dma_start(wq_sb[:, k * NQ:(k + 1) * NQ],
                                        wq_d[:, k * NQ:(k + 1) * NQ])
                for k in range(KT):
                    nc.scalar.dma_start(wkv_sb[:, k * NKVW:(k + 1) * NKVW],
                                        wkv_d[:, k * NKVW:(k + 1) * NKVW])
                nc.gpsimd.dma_start(hsT0[:, 2 * P:8 * P], hsT_d[0:P, 2 * P:8 * P])
                nc.gpsimd.dma_start(hsT0[:, 8 * P:], hsT_d[0:P, 8 * P:])
                for k in range(2, KT):
                    nc.sync.dma_start(wq_sb[:, k * NQ:(k + 1) * NQ],
                                      wq_d[:, k * NQ:(k + 1) * NQ])
                for k in range(KT):
                    sl = slice(k * HD, (k + 1) * HD)
                    nc.sync.dma_start(cq_sb[:, sl], cq_d[:, sl])
                    nc.sync.dma_start(sq_sb[:, sl], sq_d[:, sl])
                    nc.sync.dma_start(ck_sb[:, sl], ck_d[:, sl])
                    nc.sync.dma_start(sk_sb[:, sl], sk_d[:, sl])
                nc.sync.dma_start(wo_sb[:], wo_d[:])

                def flush_tr(st):
                    t_ = st["t"]
                    # vS copy first: it releases the pskv PSUM buffer and has
                    # no transpose dependency
                    nc.vector.tensor_copy(
                        vS_r[:, :, t_ * P:(t_ + 1) * P],
                        st["pskv"][:, KV * HD:2 * KV * HD].rearrange(
                            "p (h s) -> p h s", h=KV))
                    ptr = ps_tr.tile([P, NJ * P], bf16, tag="ptr", name="ptr")
                    for grp, nh, j0 in (("q", HEADS, 0), ("k", KV, HEADS)):
                        qr = st["qrots"][grp]
                        for i in range(nh):
                            # transpose-mode matmul: the only PE path that can
                            # write bf16 to PSUM (keeps ptr at 1 bank and the
                            # DVE copies in 2x mode)
                            nc.tensor.transpose(
                                ptr[:, (j0 + i) * P:(j0 + i + 1) * P],
                                qr[:, i * HD:(i + 1) * HD], ident[:])
                    nc.vector.tensor_copy(
                        qT_r[:, :, t_ * P:(t_ + 1) * P],
                        ptr[:, 0:HEADS * P].rearrange("p (h s) -> p h s", h=HEADS))
                    nc.vector.tensor_copy(
                        kT_r[:, :, t_ * P:(t_ + 1) * P],
                        ptr[:, HEADS * P:NJ * P].rearrange("p (h s) -> p h s", h=KV))

                # early attention for head (c0,h0): score group g needs only
                # k-tiles 2g,2g+1 (flushed by tile 2g+2's k==12 slot) and qT
                # tiles 0..3 (flushed by tile 4), so groups 0..6 are emitted
                # INSIDE the t-loop right after the flush slot, where the two
                # extra matmuls ride the PE-bound stream and the exps hide
                # under ACT's large stage-1 slack. This starts the stage-2
                # exp stream (chunk 0's pacer) a full head early and leaves
                # no serial exp ping-pong at the stage boundary.
                e_early = ep0.tile([P, NG * GW], bf16, name="e_early")
                # head (c0,h1)'s first four groups ride the remaining odd
                # tiles; its last four run normally in stage 2, halving chunk
                # 0's ACT-exp deficit
                e_early2 = ep0.tile([P, 4 * GW], bf16, name="e_early2")
                EARLY_AT = {5: (0, 0), 6: (0, 1), 7: (0, 2), 8: (0, 3),
                            10: (0, 4), 12: (0, 5), 14: (0, 6),
                            9: (1, 0), 11: (1, 1), 13: (1, 2), 15: (1, 3)}

                def early_score(hh, g):
                    psc = ps_sc_a.tile([P, GW], f32, tag="psc", name="psca")
                    dst = e_early if hh == 0 else e_early2
                    for i in range(GRP):
                        kk = g * GRP + i
                        nc.tensor.matmul(
                            psc[:, i * QW:(i + 1) * QW],
                            kT_r[:, 0, kk * P:(kk + 1) * P],
                            qT_r[:, hh, 0:QW], start=True, stop=True)
                    nc.scalar.activation(dst[:, g * GW:(g + 1) * GW],
                                         psc[:], AF.Exp, scale=inv_sqrt_hd)

                pend_tr = None
                for t in range(NT):
                    if t == 0:
                        hsTt = hsT0
                    else:
                        hsTt = hst.tile([P, HID], bf16, tag="hsTt", name="hsTt")
                        nc.gpsimd.dma_start(hsTt[:], hsT_d[t * P:(t + 1) * P, :])
                    psq = ps_qkv.tile([P, NQ], f32, tag="psq")
                    pskv = ps_qkv.tile([P, NKVW], f32, tag="pskv")
                    for k in range(KT):
                        if k == 12 and pend_tr is not None:
                            # previous tile's transposes + copies, deferred
                            # into this tile's matmul stream; k==12 gives the
                            # rope chain ~5us of slack (at k==8 the first
                            # transpose stalled ~1us/tile on the rope adds)
                            flush_tr(pend_tr)
                            pend_tr = None
                        if k == 13 and t in EARLY_AT:
                            early_score(*EARLY_AT[t])
                        hk = hsTt[:, k * P:(k + 1) * P]
                        nc.tensor.matmul(psq[:], hk, wq_sb[:, k * NQ:(k + 1) * NQ],
                                         start=(k == 0), stop=(k == KT - 1))
                        nc.tensor.matmul(pskv[:], hk, wkv_sb[:, k * NKVW:(k + 1) * NKVW],
                                         start=(k == 0), stop=(k == KT - 1))

                    # sum-of-squares: one wide Square per projection group,
                    # then an inner-axis DVE reduce per head
                    sqq = rp.tile([P, NQ], f32, tag="sqq", name="sqq")
                    nc.scalar.activation(sqq[:], psq[:], AF.Square)
                    sqk = rp.tile([P, KV * HD], f32, tag="sqk", name="sqk")
                    nc.scalar.activation(sqk[:], pskv[:, 0:KV * HD], AF.Square)
                    sumsq = sm.tile([P, 8], f32, tag="sumsq")
                    nc.vector.tensor_reduce(
                        sumsq[:, 0:HEADS],
                        sqq.rearrange("p (h d) -> p h d", h=HEADS),
                        AX.X, OP.add)
                    nc.vector.tensor_reduce(
                        sumsq[:, HEADS:NJ],
                        sqk.rearrange("p (h d) -> p h d", h=KV),
                        AX.X, OP.add)
                    std = sm.tile([P, 8], f32, tag="std")
                    nc.scalar.activation(std[:, 0:NJ], sumsq[:, 0:NJ], AF.Sqrt,
                                         scale=1.0 / HD, bias=epsb[:])
                    rstd = sm.tile([P, 8], f32, tag="rstd")
                    nc.vector.reciprocal(rstd[:, 0:NJ], std[:, 0:NJ])

                    qrots = {}
                    # wide RoPE: q heads [P, 4*HD] then kv heads [P, 2*HD]
                    for grp, nh, j0, cos_sb, sin_sb in (
                            ("q", HEADS, 0, cq_sb, sq_sb),
                            ("k", KV, HEADS, ck_sb, sk_sb)):
                        W = nh * HD
                        src = psq if grp == "q" else pskv
                        qn = rp.tile([P, HEADS * HD], f32, tag=f"qn{grp}",
                                     name="qn")
                        # apply rstd as ONE DVE mul with the per-head scale
                        # stride-0-broadcast along d (vs 6 serial ~460ns ACT
                        # copies: shortens the rope-chain latency ~1.7us)
                        rstd_b = rstd[:, j0:j0 + nh].unsqueeze(2).broadcast_to(
                            [P, nh, HD])
                        nc.vector.tensor_mul(
                            qn[:, 0:W].rearrange("p (h d) -> p h d", h=nh),
                            src[:, 0:W].rearrange("p (h d) -> p h d", h=nh),
                            rstd_b)
                        cos_t = cos_sb[:, t * HD:(t + 1) * HD]
                        sin_t = sin_sb[:, t * HD:(t + 1) * HD]
                        qn_r = qn[:, 0:W].rearrange("p (h d) -> p h d", h=nh)
                        t1 = rp.tile([P, HEADS * HD], f32, tag=f"t1{grp}",
                                     name="t1")
                        nc.vector.tensor_mul(t1[:, 0:W], qn[:, 0:W], bcast(cos_t, nh))
                        t2 = rp.tile([P, HEADS * HD], f32, tag=f"t2{grp}",
                                     name="t2")
                        t2_r = t2[:, 0:W].rearrange("p (h d) -> p h d", h=nh)
                        nc.vector.tensor_mul(t2_r[:, :, 0:H2], qn_r[:, :, H2:HD],
                                             bcast(sin_t[:, 0:H2], nh))
                        nc.vector.tensor_mul(t2_r[:, :, H2:HD], qn_r[:, :, 0:H2],
                                             bcast(sin_t[:, H2:HD], nh))
                        qrot = rp.tile([P, HEADS * HD], bf16, tag=f"qr{grp}",
                                       name="qrot")
                        nc.vector.tensor_add(qrot[:, 0:W], t1[:, 0:W], t2[:, 0:W])
                        qrots[grp] = qrot
                    pend_tr = {"t": t, "qrots": qrots, "pskv": pskv}

                # the early head's PV accumulation for groups 0..6 (vS tiles
                # 0..13 are flushed): pure PE filler that covers tile 15's
                # rope-chain latency before its transposes can run
                ppv_early = ps_pv_a.tile([P, QW], f32, tag="ppv",
                                         name="ppv_early")
                for g in range(7):
                    for i in range(GRP):
                        kk = g * GRP + i
                        nc.tensor.matmul(
                            ppv_early[:], vS_r[:, 0, kk * P:(kk + 1) * P],
                            e_early[:, g * GW + i * QW:g * GW + (i + 1) * QW],
                            start=(kk == 0), stop=False)
                flush_tr(pend_tr)
                pend_tr = None
                # group 7 needs tile 15's kT, so it follows the last flush
                early_score(0, 7)

            # ---------------- stage 2: attention + O-projection -----------
            with tc.tile_pool(name="ep", bufs=2) as ep, \
                 tc.tile_pool(name="ac", bufs=2) as ac, \
                 tc.tile_pool(name="dn", bufs=2) as dn, \
                 tc.tile_pool(name="on", bufs=8) as on, \
                 tc.tile_pool(name="ob", bufs=3) as ob, \
                 tc.tile_pool(name="ps_sc", bufs=1, space="PSUM") as ps_sc, \
                 tc.tile_pool(name="ps_pv", bufs=1, space="PSUM") as ps_pv, \
                 tc.tile_pool(name="ps_o", bufs=2, space="PSUM") as ps_o:

                wo_state = {"row": None}

                def wo_part(c, onT_c, p, final=False):
                    # one [P, EW] slice of the previous chunk's O-projection:
                    # 4 accumulating matmuls + a PSUM->SBUF copy (+ DMA on
                    # row completion). Interleaved between score groups so
                    # the exp stream never starves.
                    qt, e_ = divmod(p, EC)
                    if e_ == 0:
                        wo_state["row"] = ob.tile([P, HID], bf16, tag="outb",
                                                  name="out_row")
                    out_row = wo_state["row"]
                    pso = ps_o.tile([P, EW], f32, tag="pso", name="pso")
                    for h in range(HEADS):
                        nc.tensor.matmul(
                            pso[:],
                            onT_c[h][:, qt * P:(qt + 1) * P],
                            wo_sb[:, h * HID + e_ * EW: h * HID + (e_ + 1) * EW],
                            start=(h == 0), stop=(h == HEADS - 1))
                    if final and e_ % 2 == 0:
                        # at the end-of-kernel drain ACT is idle: splitting
                        # the copies across both engines doubles throughput
                        nc.scalar.activation(out_row[:, e_ * EW:(e_ + 1) * EW],
                                             pso[:], AF.Copy)
                    else:
                        nc.vector.tensor_copy(out_row[:, e_ * EW:(e_ + 1) * EW],
                                              pso[:])
                    # stream each half-row as soon as its parts are done so
                    # the end-of-kernel DMA tail is half as long; the drain
                    # chunk goes out in quarters
                    rows = out_d[c * QW + qt * P: c * QW + (qt + 1) * P, :]
                    if e_ == 1:
                        nc.sync.dma_start(rows[:, 0:2 * EW],
                                          out_row[:, 0:2 * EW])
                    elif final and e_ >= 2:
                        nc.sync.dma_start(rows[:, e_ * EW:(e_ + 1) * EW],
                                          out_row[:, e_ * EW:(e_ + 1) * EW])
                    elif e_ == EC - 1:
                        nc.sync.dma_start(rows[:, 2 * EW:],
                                          out_row[:, 2 * EW:])

                def emit_wo(c, onT_c):
                    for p in range(QC * EC):
                        wo_part(c, onT_c, p, final=True)

                def emit_den(st):
                    # cross-partition reduce of the per-partition denominator
                    # tree root as a single ones-matmul (~0.2us on the PE, vs
                    # 3.3us for the old GPSIMD partition_all_reduce); every
                    # output partition receives the same column sum, so the
                    # result lands pre-replicated in PSUM. Shares the ps_o
                    # buffer rotation to stay within the 8 PSUM banks.
                    pden = ps_o.tile([P, QW], f32, tag="pso", name="pden")
                    nc.tensor.matmul(pden[:], ones[:], st["root"][:],
                                     start=True, stop=True)
                    st["den"] = pden

                def emit_tail(st):
                    # deferred normalization tail for a finished head:
                    # reciprocal of the broadcast denominator, then one mul
                    # that both normalizes and moves ppv PSUM->SBUF.
                    rcp = dn.tile([P, QW], f32, tag="rcp", name="rcp")
                    nc.vector.reciprocal_approx_fast(rcp[:], st["den"][:])
                    onT = on.tile([P, QW], bf16, tag="onT", name="onT")
                    nc.vector.tensor_mul(onT[:], st["ppv"][:], rcp[:])
                    st["onT_c"].append(onT)

                # O-projection part slots (head, group). Head-START slots
                # (h,0) fill the boundary where the next head's first score
                # matmul waits on the lagging exp stream; h0 only gets late
                # slots (g5/g7) because the previous chunk's h3 tail -- which
                # completes onT_c -- lands at (h0, g3).
                part_slots = {(0, 5), (0, 7), (1, 0), (2, 0), (3, 0)} | {
                    (h, g) for h in range(1, HEADS) for g in (1, 3, 5, 7)}

                prev_wo = None
                prev_parts = 0
                prev_tail = None
                prev_pv = None
                for c in range(QC):
                    onT_c = []
                    for h in range(HEADS):
                        # head (0,0)'s scores, exps and PV groups 0..6
                        # already ran during stage 1 (e_early / ppv_early);
                        # head (0,1)'s first four score groups are in e_early2
                        early = (c == 0 and h == 0)
                        half2 = (c == 0 and h == 1)
                        last = (c == QC - 1 and h == HEADS - 1)
                        kv = h // (HEADS // KV)
                        hidx = c * HEADS + h
                        # heads alternate between the hoisted 1-buf pool and
                        # the stage-2 one (= the old bufs=2 rotation)
                        ppv = ppv_early if early else (
                            ps_pv_a if hidx % 2 == 0 else ps_pv).tile(
                                [P, QW], f32, tag="ppv", name="ppv")
                        # all 8 exp groups of this head live in one wide tile
                        # so the denominator tree runs as 4 strided DVE ops
                        e_all = e_early if early else ep.tile(
                            [P, NG * GW], bf16, tag="e", name="e_all")

                        def esrc(g):
                            # which tile holds this head's exp group g
                            return e_early2 if (half2 and g < 4) else e_all

                        def pv_of(g, dst):
                            src = esrc(g)
                            for i in range(GRP):
                                kk = g * GRP + i
                                nc.tensor.matmul(
                                    dst[:], vS_r[:, kv, kk * P:(kk + 1) * P],
                                    src[:, g * GW + i * QW:g * GW + (i + 1) * QW],
                                    start=(kk == 0), stop=(kk == NT - 1))

                        halfsums = []

                        def den_half(lo):
                            # sum exp groups [lo, lo+4) pairwise with fully
                            # contiguous [P, 2048] operands (strided views
                            # break the DVE fast path); groups lo..lo+3 always
                            # live in one tile (esrc is constant over them)
                            src = esrc(lo)
                            hs_ = ac.tile([P, 2 * GW], bf16, tag="hsum",
                                          name="hs_")
                            nc.vector.tensor_add(
                                hs_[:], src[:, lo * GW:(lo + 2) * GW],
                                src[:, (lo + 2) * GW:(lo + 4) * GW])
                            halfsums.append(hs_)

                        for g in range(NG):
                            emit_sc = not early and not (half2 and g < 4)
                            if emit_sc:
                                # alternate between the hoisted 2-bank pool
                                # and the stage-2 one (= the old bufs=2 ring)
                                psc = (ps_sc_a if g % 2 == 0 else ps_sc).tile(
                                    [P, GW], f32, tag="psc", name="psc")
                            # PE work for the PREVIOUS group goes first so the
                            # score matmuls never catch up with the exp stream
                            if g >= 1:
                                if g == 1 and prev_pv is not None:
                                    # previous head's trailing PVs, deferred
                                    # far enough that their exps completed
                                    prev_pv()
                                    prev_pv = None
                                if not early and g >= 2:
                                    # PV runs TWO slots behind its score
                                    # group so the exp stream never stalls it
                                    # (the early head's groups 0..6 were
                                    # accumulated into ppv_early in stage 1)
                                    pv_of(g - 2, ppv)
                            if g == 2 and prev_tail is not None:
                                # previous head's denominator matmul, deferred
                                # far enough that its DVE tree has completed
                                emit_den(prev_tail)
                            if g == 3 and prev_tail is not None:
                                # previous head's normalization, deferred far
                                # enough that its all-reduce has completed
                                emit_tail(prev_tail)
                                prev_tail = None
                            if (prev_wo is not None and (h, g) in part_slots
                                    and prev_parts < QC * EC):
                                wo_part(prev_wo[0], prev_wo[1], prev_parts)
                                prev_parts += 1
                            if emit_sc:
                                for i in range(GRP):
                                    kk = g * GRP + i
                                    nc.tensor.matmul(
                                        psc[:, i * QW:(i + 1) * QW],
                                        kT_r[:, kv, kk * P:(kk + 1) * P],
                                        qT_r[:, h, c * QW:(c + 1) * QW],
                                        start=True, stop=True)
                                nc.scalar.activation(
                                    e_all[:, g * GW:(g + 1) * GW], psc[:],
                                    AF.Exp, scale=inv_sqrt_hd)
                            if g == 3:
                                den_half(0)
                            if last and g == 6:
                                # pre-fold everything not depending on exp6/7
                                # so the end-of-kernel tail after the final
                                # exp is as short as possible
                                dq45 = ac.tile([P, GW], bf16, tag="dq45",
                                               name="dq45")
                                nc.vector.tensor_add(dq45[:],
                                                     e_all[:, 4 * GW:5 * GW],
                                                     e_all[:, 5 * GW:6 * GW])
                                fold0 = ac.tile([P, GW], bf16, tag="fold0",
                                                name="fold0")
                                nc.vector.tensor_add(fold0[:],
                                                     halfsums[0][:, 0:GW],
                                                     halfsums[0][:, GW:2 * GW])
                                r0 = ac.tile([P, QW], bf16, tag="r0", name="r0")
                                nc.vector.tensor_add(r0[:], fold0[:, 0:QW],
                                                     fold0[:, QW:GW])
                        if last:
                            # short-critical-path tree: only ~1.8us of DVE
                            # work remains after the final exp
                            dq67 = ac.tile([P, GW], bf16, tag="dq67",
                                           name="dq67")
                            nc.vector.tensor_add(dq67[:],
                                                 e_all[:, 6 * GW:7 * GW],
                                                 e_all[:, 7 * GW:8 * GW])
                            s2 = ac.tile([P, GW], bf16, tag="s2", name="s2")
                            nc.vector.tensor_add(s2[:], dq45[:], dq67[:])
                            r1 = ac.tile([P, QW], bf16, tag="r1", name="r1")
                            nc.vector.tensor_add(r1[:], s2[:, 0:QW],
                                                 s2[:, QW:GW])
                            root = ac.tile([P, QW], bf16, tag="root",
                                           name="root")
                            nc.vector.tensor_add(root[:], r0[:], r1[:])
                        else:
                            # denominator: finish the strided bf16 tree, fold
                            # (the cross-partition reduce is the ones-matmul
                            # in emit_den)
                            den_half(4)
                            qsum = ac.tile([P, 2 * GW], bf16, tag="qsum",
                                           name="qsum")
                            nc.vector.tensor_add(qsum[:], halfsums[0][:],
                                                 halfsums[1][:])
                            wsum = ac.tile([P, GW], bf16, tag="wsum",
                                           name="wsum")
                            nc.vector.tensor_add(wsum[:], qsum[:, 0:GW],
                                                 qsum[:, GW:2 * GW])
                            root = ac.tile([P, QW], bf16, tag="root",
                                           name="root")
                            nc.vector.tensor_add(root[:], wsum[:, 0:QW],
                                                 wsum[:, QW:2 * QW])
                        tail_gs = (NG - 1,) if early else (NG - 2, NG - 1)
                        prev_pv = (lambda gs=tail_gs, dst=ppv, ea=e_all, kvv=kv:
                                   [nc.tensor.matmul(
                                       dst[:], vS_r[:, kvv, (gg * GRP + i) * P:
                                                    (gg * GRP + i + 1) * P],
                                       ea[:, gg * GW + i * QW:gg * GW + (i + 1) * QW],
                                       start=False,
                                       stop=(gg == NG - 1 and i == GRP - 1))
                                    for gg in gs for i in range(GRP)])
                        prev_tail = {"root": root, "ppv": ppv, "onT_c": onT_c}

                    assert prev_wo is None or prev_parts == QC * EC, prev_parts
                    prev_wo = (c, onT_c)
                    prev_parts = 0
                prev_pv()
                prev_pv = None
                emit_den(prev_tail)
                emit_tail(prev_tail)
                prev_tail = None
                emit_wo(*prev_wo)

    nc.compile()
    _CACHE["nc"] = nc
    return nc


def _host_prep(hidden_states, position_ids, Wq, Wk, Wv, Wo, q_norm_w, k_norm_w):
    """Build the 8 per-core input maps (bf16 matmul operands, pre-tiled)."""
    import ml_dtypes
    bf = ml_dtypes.bfloat16

    hidden_states = np.asarray(hidden_states, dtype=np.float32)
    Wq = np.asarray(Wq, dtype=np.float32)
    Wk = np.asarray(Wk, dtype=np.float32)
    Wv = np.asarray(Wv, dtype=np.float32)
    Wo = np.asarray(Wo, dtype=np.float32)
    q_norm_w = np.asarray(q_norm_w, dtype=np.float32)
    k_norm_w = np.asarray(k_norm_w, dtype=np.float32)
    pos = np.asarray(position_ids)

    ident = np.eye(P, dtype=bf)

    # per-batch rope tables with sign fold and norm-weight fold, pre-tiled
    # to [p, (t, d)] so each table is a single contiguous DMA
    inv_freq = (1.0 / THETA ** (np.arange(0, HD, 2, dtype=np.float32) / HD)
                ).astype(np.float32)
    H2 = HD // 2

    def tile_tab(x):  # [S, HD] -> [P, NT*HD] f32
        return np.ascontiguousarray(
            x.reshape(NT, P, HD).transpose(1, 0, 2).reshape(P, NT * HD))

    tabs = []
    for b in range(B):
        freqs = pos[b].astype(np.float32)[:, None] * inv_freq[None, :]
        emb = np.concatenate([freqs, freqs], axis=-1)          # [S, HD]
        cos = np.cos(emb).astype(np.float32)
        sin = np.sin(emb).astype(np.float32)
        sinx = sin.copy()
        sinx[:, :H2] *= -1.0
        wq_sw = np.concatenate([q_norm_w[H2:], q_norm_w[:H2]])
        wk_sw = np.concatenate([k_norm_w[H2:], k_norm_w[:H2]])
        tabs.append({
            "cosq": tile_tab(cos * q_norm_w[None, :]),
            "sinxq": tile_tab(sinx * wq_sw[None, :]),
            "cosk": tile_tab(cos * k_norm_w[None, :]),
            "sinxk": tile_tab(sinx * wk_sw[None, :]),
        })

    # Pre-tiled transpose: hsT_t[t*P+p, k*P+c] = hs[b][t*P+c, k*P+p] so each
    # s-tile's SBUF load is a plain [P, HID] slice with 4KB-contiguous rows.
    hsT = []
    for b in range(B):
        x = hidden_states[b].reshape(NT, P, KT, P)      # [t, c, k, p]
        x = np.ascontiguousarray(x.transpose(0, 3, 2, 1))  # [t, p, k, c]
        hsT.append(x.reshape(NT * P, KT * P).astype(bf))

    def tile_w(w):  # [HID, N] -> [P, KT*N]
        n = w.shape[1]
        return np.ascontiguousarray(
            w.reshape(KT, P, n).transpose(1, 0, 2).reshape(P, KT * n)).astype(bf)

    def tile_wo(w):  # [HEADS*HD, HID] -> [P, HEADS*HID]
        return np.ascontiguousarray(
            w.reshape(HEADS, P, HID).transpose(1, 0, 2).reshape(P, HEADS * HID)
        ).astype(bf)

    in_maps = []
    for c in range(N_CORES):
        b = c // 4
        q = c % 4
        qs = slice(q * HEADS * HD, (q + 1) * HEADS * HD)
        ks = slice(q * KV * HD, (q + 1) * KV * HD)
        in_maps.append({
            "hsT": hsT[b],
            "wq": tile_w(Wq[:, qs]),
            "wkv": tile_w(np.concatenate([Wk[:, ks], Wv[:, ks]], axis=1)),
            "wo": tile_wo(Wo[qs, :]),
            "cosq": tabs[b]["cosq"],
            "sinxq": tabs[b]["sinxq"],
            "cosk": tabs[b]["cosk"],
            "sinxk": tabs[b]["sinxk"],
            "ident": ident,
        })
    return in_maps


def _gather(results):
    out = np.empty((B, S, HID), dtype=np.float32)
    for b in range(B):
        acc = results[4 * b]["out"].astype(np.float32)
        for i in range(1, 4):
            acc = acc + results[4 * b + i]["out"].astype(np.float32)
        out[b] = acc
    return out


def kernel(hidden_states, position_ids, Wq, Wk, Wv, Wo, q_norm_w, k_norm_w,
           _trace=False):
    from concourse.bass_utils import run_bass_kernel_spmd

    nc = _build()
    in_maps = _host_prep(hidden_states, position_ids, Wq, Wk, Wv, Wo,
                         q_norm_w, k_norm_w)
    res = run_bass_kernel_spmd(nc, in_maps, core_ids=list(range(N_CORES)),
                               trace=_trace)
    out = _gather(res.results)
    if _trace:
        kernel.last_result = res
    return out

